# revision 68
# baseline (speedup 1.0000x reference)
"""Multi-head attention (B=2, S=2048, D=1024, H=16, dk=64) on 8 TRN2 cores.

Sharding: core c -> (batch b = c//4, head-group g = c%4 of 4 heads).
Each core computes q/k/v projections for its 4 heads, full attention for
those heads, and a partial output projection (rows g*256:(g+1)*256 of Wo).
Host pre-transposes/casts inputs to bf16 and sums the partial outputs.

Cost-model-driven layout (PE matmul cost = out-free-size x chunks):
  scoresT[j, i] in PSUM ([128, 2h, 512] per (pair, iq, jt)); exp on ACT
  (one [128, 1024] instr per jt -> probsT bf16; no max-subtract: scores
  ~ N(0,1) after 1/8 scaling).
  PV "orientation B": attn_u[i, e] = sum_j probsT[j,i] * v_aug[j,e] with
  probsT tiles as stationary weights -> 65 rows per matmul instead of 512
  (i on partitions). v_aug = [v_h | ones]; col 64 = softmax denominator.
  Normalize on DVE (reciprocal + broadcast tensor_mul, cast bf16), then
  PE-transpose [128 i, 128 e] blocks back to attnT[e, s] for out-proj.
  out-projT: out[s, n] = sum_c attnT_c[:, s].T @ wo_c (K=128 x2).
Schedule: pair-major pass order (all 4 i-blocks of pair 0, then pair 1)
over flattened (i-block, pair, jt) steps; scores/exp emitted one step
ahead of the trailing PV so ACT (the bottleneck: 128 x 1038ns exp) stays
saturated. Pair 0's k/v projections are JIT'd 2 steps ahead inside the
first (DMA-paced) pass; pair 1's projections, later q-proj 128-col parts,
and the transpose/out-proj tails are deadline-ordered filler units pumped
into the ACT-bound slack of later passes (min_g gates each unit until its
DMA block has landed so a pumped unit never head-blocks the PE queue).
PE p-state warm-up matmuls run during the initial DMA wait. In the final
drain, po evictions alternate DVE/ACT and use sc-tag PSUM slots.
PSUM: sc 2x2 + pv 2 + work 2 = 8 banks.
Host: out[b] = sum_g outT_partial + (bv @ Wo + bo).
"""

import os

import numpy as np
import ml_dtypes

BF16 = ml_dtypes.bfloat16

B, S, D = 2, 2048, 1024
H, DK = 16, 64
P = 128
GROUPS = 4          # head groups (one per core within a batch)
HPG = 4             # heads per group
GD = HPG * DK       # 256, group width
KC = D // P         # 8 contraction chunks
NJT = S // P        # 16 j-tiles
NIQ = S // 512      # 4 i-blocks of 512
NCORES = 8

_cached = {}


def _build_bass():
    import concourse.bass as bass
    import concourse.tile as tile
    from concourse.bacc import Bacc
    from concourse import mybir
    from contextlib import ExitStack

    f32 = mybir.dt.float32
    bf16 = mybir.dt.bfloat16
    Act = mybir.ActivationFunctionType

    nc = Bacc()

    xqT = nc.dram_tensor("xqT", [D, S], bf16, kind="ExternalInput")
    xkT = nc.dram_tensor("xkT", [D, S], bf16, kind="ExternalInput")
    xvT = nc.dram_tensor("xvT", [D, S], bf16, kind="ExternalInput")
    wq = nc.dram_tensor("wq", [D, GD], bf16, kind="ExternalInput")
    wk = nc.dram_tensor("wk", [D, GD], bf16, kind="ExternalInput")
    wv = nc.dram_tensor("wv", [D, GD], bf16, kind="ExternalInput")
    wo = nc.dram_tensor("wo", [GD, D], bf16, kind="ExternalInput")
    bq = nc.dram_tensor("bq", [GD, 1], f32, kind="ExternalInput")
    bk = nc.dram_tensor("bk", [GD, 1], f32, kind="ExternalInput")
    ident = nc.dram_tensor("ident", [P, P], bf16, kind="ExternalInput")
    out = nc.dram_tensor("out", [S, D], bf16, kind="ExternalOutput")

    with tile.TileContext(nc) as tc, ExitStack() as ctx:
        singles = ctx.enter_context(tc.tile_pool(name="singles", bufs=1))
        probs_pool = ctx.enter_context(tc.tile_pool(name="probs", bufs=4))
        pre_pool = ctx.enter_context(tc.tile_pool(name="pre", bufs=4))
        small = ctx.enter_context(tc.tile_pool(name="small", bufs=4))
        outs_pool = ctx.enter_context(tc.tile_pool(name="outs", bufs=4))
        psum = ctx.enter_context(tc.tile_pool(name="psum", bufs=1, space="PSUM"))

        # ---- persistent SBUF ----
        wq_sb = singles.tile([P, KC, GD], bf16)
        wk_sb = singles.tile([P, KC, GD], bf16)
        wv_sb = singles.tile([P, KC, GD], bf16)
        wo_sb = singles.tile([P, 2, D], bf16)
        bq_sb = singles.tile([P, 2, 1], f32)
        bk_sb = singles.tile([P, 2, 1], f32)
        ident_sb = singles.tile([P, P], bf16)
        xq_sb = singles.tile([P, KC, S], bf16)
        xk_sb = singles.tile([P, KC, S], bf16)
        xv_sb = singles.tile([P, KC, S], bf16)
        qT = [singles.tile([P, S], bf16, name=f"qT{t}") for t in range(2)]
        kT = [singles.tile([P, S], bf16, name=f"kT{t}") for t in range(2)]
        attT = [singles.tile([P, S], bf16, name=f"attT{t}") for t in range(2)]
        # v_aug per (jt, head): [v | ones]; ones col -> softmax denominator
        v_sb = singles.tile([P, NJT, HPG, 65], bf16)
        nc.vector.memset(v_sb[:, :, :, 64:65], 1.0)
        # PE p-state warm-up: dummy matmuls on a zeroed tile keep the PE
        # continuously busy through the initial DMA wait, so the real
        # projections start at full clock (2.4 GHz needs 3us of busy ramp)
        wu = singles.tile([P, P], bf16)
        nc.vector.memset(wu, 0.0)

        # ---- DMA emission order = DMA-engine service order ----
        # First-exp critical path: wq, xq i-block 0 (two 256-col halves),
        # wk, xk j-block 0; then xk/xv 256-col j-blocks chased by the JIT
        # k/v projections during the first i-block pass.
        def dma_x(dst, src, c0, c1):
            nc.sync.dma_start(
                out=dst[:, :, c0:c1],
                in_=src[:, c0:c1].rearrange("(c p) m -> p c m", p=P))

        nc.sync.dma_start(out=wq_sb, in_=wq.rearrange("(c p) m -> p c m", p=P))
        dma_x(xq_sb, xqT, 0, 256)
        nc.sync.dma_start(out=bq_sb, in_=bq.rearrange("(t p) o -> p t o", p=P))
        nc.sync.dma_start(out=bk_sb, in_=bk.rearrange("(t p) o -> p t o", p=P))
        dma_x(xq_sb, xqT, 256, 512)
        nc.sync.dma_start(out=wk_sb, in_=wk.rearrange("(c p) m -> p c m", p=P))
        dma_x(xk_sb, xkT, 0, 256)
        nc.sync.dma_start(out=wv_sb, in_=wv.rearrange("(c p) m -> p c m", p=P))
        dma_x(xv_sb, xvT, 0, 256)
        for m in range(1, 8):
            dma_x(xk_sb, xkT, m * 256, (m + 1) * 256)
            dma_x(xv_sb, xvT, m * 256, (m + 1) * 256)
            if m == 4:
                dma_x(xq_sb, xqT, 512, 1024)
        nc.sync.dma_start(out=ident_sb, in_=ident[:, :])
        for b_ in range(2, 4):
            dma_x(xq_sb, xqT, b_ * 512, (b_ + 1) * 512)
        nc.sync.dma_start(out=wo_sb, in_=wo.rearrange("(c p) n -> p c n", p=P))

        # ---- projection emitters (PSUM "work" slots, 2 rotating banks) ----
        # q-proj is emitted as 4 independent 128-column parts (each a
        # complete 8-chunk accumulation + bias) so it can spread across
        # steps without holding a work slot across other users.
        def emit_qproj_part(p, iqb, part):
            w = psum.tile([P, 512], f32, tag="work", bufs=2, name="wq_ps")
            c0 = iqb * 512 + part * P
            for k in range(KC):
                nc.tensor.matmul(
                    out=w[:, 0:P],
                    lhsT=wq_sb[:, k, p * P:(p + 1) * P],
                    rhs=xq_sb[:, k, c0:c0 + P],
                    start=(k == 0), stop=(k == KC - 1))
            nc.vector.tensor_scalar_add(
                out=qT[p][:, c0:c0 + P], in0=w[:, 0:P],
                scalar1=bq_sb[:, p, :])

        def emit_qproj(p, iqb):
            for part in range(4):
                emit_qproj_part(p, iqb, part)

        def emit_kproj(p, jt):
            w = psum.tile([P, 512], f32, tag="work", bufs=2, name="wk_ps")
            for k in range(KC):
                nc.tensor.matmul(
                    out=w[:, 0:P],
                    lhsT=wk_sb[:, k, p * P:(p + 1) * P],
                    rhs=xk_sb[:, k, jt * P:(jt + 1) * P],
                    start=(k == 0), stop=(k == KC - 1))
            nc.vector.tensor_scalar_add(
                out=kT[p][:, jt * P:(jt + 1) * P], in0=w[:, 0:P],
                scalar1=bk_sb[:, p, :])

        def emit_vproj(p, jt):
            w = psum.tile([P, 512], f32, tag="work", bufs=2, name="wv_ps")
            for k in range(KC):
                nc.tensor.matmul(
                    out=w[:, 0:P],
                    lhsT=xv_sb[:, k, jt * P:(jt + 1) * P],
                    rhs=wv_sb[:, k, p * P:(p + 1) * P],
                    start=(k == 0), stop=(k == KC - 1))
            nc.vector.tensor_copy(
                out=v_sb[:, jt, 2 * p:2 * p + 2, 0:64],
                in_=w[:, 0:P].rearrange("p (h d) -> p h d", h=2))


        # ---- attention step pieces ----
        def scores_exp(iq, p, jt):
            sc = psum.tile([P, 2, 512], f32, tag="sc", bufs=2, name="sc")
            for hp in range(2):
                nc.tensor.matmul(
                    out=sc[:, hp, :],
                    lhsT=kT[p][hp * 64:(hp + 1) * 64, jt * P:(jt + 1) * P],
                    rhs=qT[p][hp * 64:(hp + 1) * 64,
                              iq * 512:(iq + 1) * 512],
                    start=True, stop=True)
            probs = probs_pool.tile([P, 2, 512], bf16, tag="probs",
                                    name="probs")
            nc.scalar.activation(out=probs, in_=sc, func=Act.Exp, scale=0.125)
            return probs

        def emit_pv(p, jt, probs, pv):
            for it in range(4):
                for hp in range(2):
                    s_ = 2 * it + hp
                    nc.tensor.matmul(
                        out=pv[:, s_ // 4, s_ % 4, 0:65],
                        lhsT=probs[:, hp, it * P:(it + 1) * P],
                        rhs=v_sb[:, jt, 2 * p + hp, :],
                        start=(jt == 0 and s_ % 4 == 0),
                        stop=(jt == NJT - 1 and s_ % 4 == 3))

        def normalize(p, pv, att_pre, split=False):
            r = small.tile([P, 8], f32, tag="r", name="r")
            nc.vector.reciprocal(
                out=r, in_=pv[:, :, :, 64:65].rearrange("p b j o -> p (b j o)"))
            if split:
                # final pass: per-bank muls so the tail transposes of the
                # first two i-tiles start before the second mul finishes
                for b_ in range(2):
                    nc.vector.tensor_mul(
                        out=att_pre[:, 2 * b_:2 * b_ + 2, p, :, :],
                        in0=pv[:, b_, :, 0:64].rearrange(
                            "p (i h) d -> p i h d", i=2),
                        in1=r[:, 4 * b_:4 * b_ + 4].rearrange(
                            "p (x h) -> p x h", h=2).to_broadcast(
                            [P, 2, 2, 64]))
            else:
                nc.vector.tensor_mul(
                    out=att_pre[:, :, p, :, :],
                    in0=pv[:, :, :, 0:64].rearrange(
                        "p b (i h) d -> p (b i) h d", i=2),
                    in1=r.rearrange("p (x h) -> p x h", h=2).to_broadcast(
                        [P, 4, 2, 64]))

        # ---- tail units: PE-transpose att_pre -> attT, then out-proj ----
        # tailA (after pair-0 normalize): transpose pair-0 blocks; tailB
        # (after pair-1): transpose pair-1 blocks + the 8 out-proj units.
        fillers = []

        def tp_unit(iq, att_pre, pr):
            def emit(final):
                tpp = psum.tile([P, 4, P], bf16, tag="work", bufs=2,
                                name="tpp")
                for it in range(4):
                    nc.tensor.matmul(
                        out=tpp[:, it, :],
                        lhsT=att_pre[:, it, pr, :, :].rearrange(
                            "p h d -> p (h d)"),
                        rhs=ident_sb, is_transpose=True,
                        start=True, stop=True)
                nc.vector.tensor_copy(
                    out=attT[pr][:, iq * 512:iq * 512 + 256],
                    in_=tpp[:, 0:2, :])
                nc.vector.tensor_copy(
                    out=attT[pr][:, iq * 512 + 256:(iq + 1) * 512],
                    in_=tpp[:, 2:4, :])
            return emit

        osb_box = {}

        def po_unit(iq, it, nb, k):
            def emit(final):
                if final and k % 2 == 0:
                    po = psum.tile([P, 2, 512], f32, tag="sc", bufs=2,
                                   name="po_sc")[:, 0, :]
                else:
                    po = psum.tile([P, 512], f32, tag="work", bufs=2,
                                   name="po")
                st = iq * 4 + it
                for c in range(2):
                    nc.tensor.matmul(
                        out=po,
                        lhsT=attT[c][:, st * P:(st + 1) * P],
                        rhs=wo_sb[:, c, nb * 512:(nb + 1) * 512],
                        start=(c == 0), stop=(c == 1))
                if nb == 0:
                    osb_box[st] = outs_pool.tile([P, 2, 512], bf16,
                                                 tag="osb", bufs=4,
                                                 name="osb")
                osb = osb_box[st]
                # in the final drain ACT is idle: alternate evictions
                if final and k % 2 == 1:
                    nc.scalar.copy(out=osb[:, nb, :], in_=po)
                else:
                    nc.vector.tensor_copy(out=osb[:, nb, :], in_=po)
                if nb == 1:
                    nc.sync.dma_start(
                        out=out[st * P:(st + 1) * P, :],
                        in_=osb_box.pop(st))
            return emit

        def enqueue_tailA(iq, att_pre):
            fillers.append((0, tp_unit(iq, att_pre, 0)))

        def enqueue_tailB(iq, att_pre):
            fillers.append((0, tp_unit(iq, att_pre, 1)))
            k = 0
            for it in range(4):
                for nb in range(2):
                    fillers.append((0, po_unit(iq, it, nb, k)))
                    k += 1

        def pump(n, final=False, g=10 ** 9):
            for _ in range(n):
                if fillers and fillers[0][0] <= g:
                    fillers.pop(0)[1](final)

        # ---- main schedule ----
        # Pair-major pass order: all four i-blocks for pair 0, then pair 1.
        # Pair 1's k/v projections and the later q-proj parts become filler
        # units drained into the ACT-bound slack of passes 2-8 (min_g gates
        # a unit until its DMA block has landed, so a pumped unit never
        # head-blocks the PE queue). scores+exp for step g+1 are emitted at
        # iteration g and the PV for step g-1 trails at iteration g, so in
        # PE program order scores(g+2) sits directly behind pv(g).
        passes = [(iq, 0) for iq in range(NIQ)] + \
                 [(iq, 1) for iq in range(NIQ)]
        steps = [(iq, p, jt) for (iq, p) in passes for jt in range(NJT)]
        wu_ps = psum.tile([P, 512], f32, tag="work", bufs=2, name="wu_ps")
        for _ in range(75):
            nc.tensor.matmul(out=wu_ps[:, 0:P], lhsT=wu, rhs=wu,
                             start=True, stop=True)
        emit_qproj(0, 0)
        emit_kproj(0, 0)
        emit_kproj(0, 1)
        emit_qproj(1, 0)
        emit_kproj(1, 0)
        emit_kproj(1, 1)

        def qproj_unit(p_, iqb, part):
            return lambda final: emit_qproj_part(p_, iqb, part)

        def kproj_unit(p_, jt_):
            return lambda final: emit_kproj(p_, jt_)

        def vproj_unit(p_, jt_):
            return lambda final: emit_vproj(p_, jt_)

        for part in range(4):
            fillers.append((10, qproj_unit(0, 1, part)))
        for part in range(4):
            fillers.append((18, qproj_unit(0, 2, part)))
        for part in range(4):
            fillers.append((24, qproj_unit(0, 3, part)))
        fillers.append((28, vproj_unit(1, 0)))
        fillers.append((28, vproj_unit(1, 1)))
        for jt_ in range(2, NJT):
            if jt_ < 6:
                mg = 30
            elif jt_ < 10:
                mg = 46
            elif jt_ < 14:
                mg = 56
            else:
                mg = 66
            fillers.append((mg, kproj_unit(1, jt_)))
            fillers.append((mg, vproj_unit(1, jt_)))
        for b_ in range(1, 4):
            for part in range(4):
                fillers.append((56 + 16 * b_, qproj_unit(1, b_, part)))

        probs_q = {}
        pv = None
        att_pre_map = {}
        pending_norm = None
        for g in range(len(steps) + 1):
            if g >= 1:
                iqp, pp, jtp = steps[g - 1]
                if jtp == 0:
                    pv = psum.tile([P, 2, 4, P], f32, tag="pv", bufs=1,
                                   name="pv")
                emit_pv(pp, jtp, probs_q.pop(g - 1), pv)
                if jtp == NJT - 1:
                    pending_norm = (pp, pv, iqp)
            if g == len(steps):
                if pending_norm is not None:
                    pp, pvn, iqn = pending_norm
                    normalize(pp, pvn, att_pre_map[iqn])
                    enqueue_tailB(iqn, att_pre_map[iqn])
                break
            iq, p, jt = steps[g]
            if g == 0:
                probs_q[0] = scores_exp(*steps[0])
            if g + 1 < len(steps):
                probs_q[g + 1] = scores_exp(*steps[g + 1])
            if iq == 0 and p == 0:
                emit_vproj(0, jt)
                if jt + 2 < NJT:
                    emit_kproj(0, jt + 2)
            if pending_norm is not None:
                pp, pvn, iqn = pending_norm
                if pp == 0:
                    att_pre_map[iqn] = pre_pool.tile(
                        [P, 4, 2, 2, 64], bf16, tag="pre", name="att_pre")
                normalize(pp, pvn, att_pre_map[iqn])
                if pp == 0:
                    enqueue_tailA(iqn, att_pre_map[iqn])
                else:
                    enqueue_tailB(iqn, att_pre_map[iqn])
                pending_norm = None
            if 2 <= jt < 16:
                pump(2 if jt % 2 == 0 else 1, g=g)
        while fillers:
            pump(1, final=True)

    nc.finalize()
    return nc


def kernel(Q, K, V, Wq, bq, Wk, bk, Wv, bv, Wo, bo):
    from concourse.bass_utils import run_bass_kernel_spmd

    f32 = np.float32
    Q = np.asarray(Q, f32)
    K = np.asarray(K, f32)
    V = np.asarray(V, f32)
    Wq = np.asarray(Wq, f32)
    Wk = np.asarray(Wk, f32)
    Wv = np.asarray(Wv, f32)
    Wo = np.asarray(Wo, f32)
    bq = np.asarray(bq, f32)
    bk = np.asarray(bk, f32)
    bv = np.asarray(bv, f32)
    bo = np.asarray(bo, f32)

    xT = {}
    for b in range(B):
        xT[('q', b)] = np.ascontiguousarray(Q[b].T).astype(BF16)
        xT[('k', b)] = np.ascontiguousarray(K[b].T).astype(BF16)
        xT[('v', b)] = np.ascontiguousarray(V[b].T).astype(BF16)
    ident_np = np.eye(P, dtype=BF16)

    in_maps = []
    for c in range(NCORES):
        b, g = c // GROUPS, c % GROUPS
        sl = slice(g * GD, (g + 1) * GD)
        in_maps.append({
            "xqT": xT[('q', b)],
            "xkT": xT[('k', b)],
            "xvT": xT[('v', b)],
            "wq": np.ascontiguousarray(Wq[:, sl]).astype(BF16),
            "wk": np.ascontiguousarray(Wk[:, sl]).astype(BF16),
            "wv": np.ascontiguousarray(Wv[:, sl]).astype(BF16),
            "wo": np.ascontiguousarray(Wo[sl, :]).astype(BF16),
            "bq": np.ascontiguousarray(bq[sl].reshape(GD, 1)),
            "bk": np.ascontiguousarray(bk[sl].reshape(GD, 1)),
            "ident": ident_np,
        })

    if "nc" not in _cached:
        _cached["nc"] = _build_bass()
    nc = _cached["nc"]

    try:
        res = run_bass_kernel_spmd(nc, in_maps, core_ids=list(range(NCORES)))
    except ModuleNotFoundError:
        # BASS_TRACE set but the axon ntff hook isn't shipped in this
        # container - retry untraced
        os.environ["BASS_NEVER_TRACE"] = "1"
        res = run_bass_kernel_spmd(nc, in_maps, core_ids=list(range(NCORES)))
    if res.exec_time_ns is not None:
        print(f"HW exec time: {res.exec_time_ns} ns")

    bo_eff = (bv @ Wo + bo).astype(f32)
    out = np.zeros((B, S, D), f32)
    for c in range(NCORES):
        b = c // GROUPS
        out[b] += np.asarray(res.results[c]["out"], f32)
    out += bo_eff
    return out
